# revision 10
# baseline (speedup 1.0000x reference)
"""Graphformer layer (full multi-head attention) on 8 trn2 NeuronCores.

Sharding: one head per core (tensor parallel over the 8 heads).

v3 design (ACT-engine-bound):
  - All matmul operands are bf16 (4x PE throughput vs fp32; tolerance is 2e-2).
  - Wo is folded into the V projection on the host: V' = x @ (Wo_h Wv_h)^T, so
    out_h = softmax(S) @ V' and no on-chip output projection is needed.
  - Scores are computed transposed (keys on partitions, queries on free dim).
    The K=64 contraction only fills half the PE array, so two key tiles are
    computed CONCURRENTLY via row-group packing: Q^T and K^T are materialized
    with duplicated partition halves (rows 0-63 == rows 64-127, free via
    host-duplicated projection weights), and even/odd key-tile matmuls use
    partitions 0:64 / 64:128 -> tile_position (0,0) / (64,0).
  - exp (scale=1/8 folded in) runs on the scalar engine from a 6-bank PSUM
    score ring in TRIPLES ([128, 3, 512] per instruction) to amortize the
    per-instruction overhead.  ACT is the bottleneck engine (~16.8M exps).
  - PV matmuls for exp-triple T are emitted AFTER the scores of triple T+1 so
    the strict-FIFO PE queue never blocks on the scalar engine (keeps the PE
    dense -> HAM stays at full clock).
  - The Q projection for group g+1 is computed mid-group-g (no boundary stall).
  - PV accumulates [65, 512] per query group in PSUM; row 64 (ones column of
    V') is the softmax denominator.  Normalization, transpose, head-sum and
    bias happen on the host from the [65, 4096] per-core output.

PSUM budget (8 banks): score ring 6 + PV accumulator 1 + Q-proj scratch 1.
"""

from contextlib import ExitStack

import numpy as np
import ml_dtypes

import concourse.bass as bass
import concourse.bacc as bacc
import concourse.mybir as mybir
from concourse.tile import TileContext

N = 4096
C = 512  # input feature dim
D = 64  # head dim
Da = D + 1  # head dim + denominator column
HEADS = 8
P = 128
F = 512  # query group width (== one PSUM bank of fp32)
NT = N // P  # 32 key tiles
CT = C // P  # 4 contraction tiles
NG = N // F  # 8 query groups
RING = 6  # PSUM score-ring depth (banks)
F32 = mybir.dt.float32
BF16 = mybir.dt.bfloat16
BF16_NP = ml_dtypes.bfloat16


def build_nc():
    nc = bacc.Bacc()
    xT = nc.declare_dram_parameter("xT", [C, N], BF16, isOutput=False)
    # weights pre-arranged on host as [P, CT*w]: w2[p, c*w+m] = W^T[c*P+p, m]
    wqd = nc.declare_dram_parameter("wqd", [P, CT * P], BF16, isOutput=False)
    wkd = nc.declare_dram_parameter("wkd", [P, CT * P], BF16, isOutput=False)
    wv2 = nc.declare_dram_parameter("wv2", [P, CT * D], BF16, isOutput=False)
    ot = nc.declare_dram_parameter("ot", [Da, N], F32, isOutput=True)

    with TileContext(nc) as tc, ExitStack() as ctx:
        const = ctx.enter_context(tc.tile_pool(name="const", bufs=1))
        sb = ctx.enter_context(tc.tile_pool(name="sb", bufs=1))
        ot_pool = ctx.enter_context(tc.tile_pool(name="otp", bufs=2))
        ps_ring = ctx.enter_context(tc.tile_pool(name="psR", bufs=1, space="PSUM"))
        ps_a = ctx.enter_context(tc.tile_pool(name="psA", bufs=1, space="PSUM"))

        # ---- input DMAs: weights first (single DMA each), then x in halves
        w_sb = {}
        for name, dram, w in (("q", wqd, P), ("k", wkd, P), ("v", wv2, D)):
            t = const.tile([P, CT, w], BF16, tag=f"w{name}")
            nc.sync.dma_start(out=t, in_=dram[:, :])
            w_sb[name] = t
        xt = [
            sb.tile([P, N], BF16, tag=f"xt{c}", name=f"xt{c}") for c in range(CT)
        ]
        for q in range(4):
            cs = slice(q * (N // 4), (q + 1) * (N // 4))
            for c in range(CT):
                eng = nc.sync if (q * CT + c) % 2 == 0 else nc.gpsimd
                eng.dma_start(out=xt[c][:, cs], in_=xT[c * P : (c + 1) * P, cs])

        # persistent SBUF tensors
        qT = sb.tile([P, N], BF16, tag="qT")  # duplicated halves
        kT = sb.tile([P, N], BF16, tag="kT")  # duplicated halves
        v_sb = sb.tile([P, NT, Da], BF16, tag="v")
        es_a = sb.tile([P, RING // 2, F], BF16, tag="esa")
        es_b = sb.tile([P, RING // 2, F], BF16, tag="esb")
        es_ab = [es_a, es_b]
        # Two separate 3-bank score tensors: Tile's PSUM dep tracking is
        # per-tensor, so a single 6-slot ring serializes scores against exp.
        # With A/B tensors, exp(X) reads one tensor while the next triple's
        # scores write the other.
        ss_a = ps_ring.tile([P, RING // 2, F], F32, tag="ssa")  # 3 banks
        ss_b = ps_ring.tile([P, RING // 2, F], F32, tag="ssb")  # 3 banks
        ss_ab = [ss_a, ss_b]

        def s_slot(t):
            return ss_ab[(t // 3) % 2][:, t % 3, :]

        def es_slot(t):
            return es_ab[(t // 3) % 2][:, t % 3, :]

        nc.vector.memset(v_sb[:, :, D:Da], 1.0)

        # warm the ACT exp table load during the projection phase
        dummy = const.tile([1, 1], F32, tag="dummy")
        nc.vector.memset(dummy, 0.0)
        nc.scalar.activation(
            out=dummy, in_=dummy, func=mybir.ActivationFunctionType.Exp
        )

        ring_i = 0

        def proj_chunk(dst, w, ch, pp):
            """Project one 512-col chunk of Q^T or K^T via psum tile pp."""
            cs = slice(ch * F, (ch + 1) * F)
            for c in range(CT):
                nc.tensor.matmul(
                    pp, w[:, c, :], xt[c][:, cs], start=(c == 0), stop=(c == CT - 1)
                )
            nc.vector.tensor_copy(out=dst[:, cs], in_=pp)

        def ring_slot():
            nonlocal ring_i
            s = ss_ab[ring_i % 2][:, (ring_i // 2) % 3, :]
            ring_i += 1
            return s

        def v_batch(b):
            """Project 8 V' tiles (keys on partitions) via one ring slot."""
            pv = ring_slot()
            for i in range(8):
                mt = b * 8 + i
                ms = slice(mt * P, (mt + 1) * P)
                for c in range(CT):
                    nc.tensor.matmul(
                        pv[:, i * D : (i + 1) * D],
                        xt[c][:, ms],
                        w_sb["v"][:, c, :],
                        start=(c == 0),
                        stop=(c == CT - 1),
                    )
            # free sizes match (512); element order (mt-major, then d) matches
            nc.vector.tensor_copy(out=v_sb[:, b * 8 : (b + 1) * 8, 0:D], in_=pv)

        # ---- projections: K + V interleaved per DMA quarter, then Q chunk 0
        for q in range(4):
            proj_chunk(kT, w_sb["k"], 2 * q, ring_slot())
            proj_chunk(kT, w_sb["k"], 2 * q + 1, ring_slot())
            v_batch(q)
        proj_chunk(qT, w_sb["q"], 0, ring_slot())


        # ---- attention, pipelined over 256 score tiles of [128 keys, 512 q]
        T = NG * NT  # 256
        po = [None] * NG
        pending_pv = []  # PV matmuls lagged one exp-triple behind scores

        def emit_pv(t):
            g, kt = divmod(t, NT)
            nc.tensor.matmul(
                po[g],
                v_sb[:, kt, :],
                es_slot(t),
                start=(kt == 0),
                stop=(kt == NT - 1),
            )
            if kt == NT - 1:
                ot_t = ot_pool.tile([Da, F], F32, tag="ot")
                for hh in range(2):
                    fs = slice(hh * (F // 2), (hh + 1) * (F // 2))
                    nc.vector.tensor_copy(out=ot_t[:, fs], in_=po[g][:, fs])
                    nc.sync.dma_start(
                        out=ot[:, g * F + hh * (F // 2) : g * F + (hh + 1) * (F // 2)],
                        in_=ot_t[:, fs],
                    )

        def emit_exp(t0, cnt):
            ab = (t0 // 3) % 2
            nc.scalar.activation(
                out=es_ab[ab][:, 0:cnt, :],
                in_=ss_ab[ab][:, 0:cnt, :],
                func=mybir.ActivationFunctionType.Exp,
                scale=0.125,
            )
            pending_pv.extend(range(t0, t0 + cnt))

        def flush_pv():
            while pending_pv:
                emit_pv(pending_pv.pop(0))

        for t in range(T):
            g, kt = divmod(t, NT)
            if kt == 0:
                po[g] = ps_a.tile([Da, F], F32, tag="po", name=f"po{g}")
            # scores: even tiles use partitions 0:64 (PE rows 0-63), odd tiles
            # 64:128 (rows 64-127) -> consecutive matmuls run concurrently.
            h0 = (t % 2) * D
            hs = slice(h0, h0 + D)
            nc.tensor.matmul(
                s_slot(t),
                kT[hs, kt * P : (kt + 1) * P],
                qT[hs, g * F : (g + 1) * F],
                start=True,
                stop=True,
            )
            if t % 3 == 2:
                flush_pv()  # PVs of the previous triple, after this triple's S
                emit_exp(t - 2, 3)
            if kt == 16 and g + 1 < NG:
                # prefetch next group's Q chunk mid-group (no boundary stall)
                pp = ps_a.tile([P, F], F32, tag="pq")
                proj_chunk(qT, w_sb["q"], g + 1, pp)
        flush_pv()
        if T % 3:
            emit_exp(T - T % 3, T % 3)
            flush_pv()

    nc.compile()
    return nc


def make_in_maps(x, Wq, Wk, Wv, Wo):
    x = np.asarray(x, dtype=np.float32)
    Wq = np.asarray(Wq, dtype=np.float32)
    Wk = np.asarray(Wk, dtype=np.float32)
    Wv = np.asarray(Wv, dtype=np.float32)
    Wo = np.asarray(Wo, dtype=np.float32)
    xT = np.ascontiguousarray(x.T).astype(BF16_NP)

    def arrange(w):  # [C, width] -> [P, CT*width]: out[p, c*w+m] = w[c*P+p, m]
        width = w.shape[1]
        return np.ascontiguousarray(
            w.reshape(CT, P, width).transpose(1, 0, 2).reshape(P, CT * width)
        ).astype(BF16_NP)

    in_maps = []
    for h in range(HEADS):
        sl = slice(h * D, (h + 1) * D)
        wqT = Wq[sl].T
        wkT = Wk[sl].T
        wv2 = (Wo[:, sl] @ Wv[sl]).T  # [C, D], output projection folded in
        in_maps.append(
            {
                "xT": xT,
                "wqd": arrange(np.concatenate([wqT, wqT], axis=1)),
                "wkd": arrange(np.concatenate([wkT, wkT], axis=1)),
                "wv2": arrange(wv2),
            }
        )
    return in_maps


_CACHE = {}


def run_on_hw(x, Wq, Wk, Wv, Wo, bo, trace=False):
    from concourse.bass_utils import run_bass_kernel_spmd

    if "nc" not in _CACHE:
        _CACHE["nc"] = build_nc()
    nc = _CACHE["nc"]
    in_maps = make_in_maps(x, Wq, Wk, Wv, Wo)
    res = run_bass_kernel_spmd(nc, in_maps, list(range(HEADS)), trace=trace)
    out = np.zeros((N, D), np.float32)
    for r in res.results:
        o = r["ot"]
        out += (o[0:D, :] / o[D:Da, :]).T
    out += np.asarray(bo, dtype=np.float32)[None, :]
    return out, res


def kernel(x, Wq, Wk, Wv, Wo, bo):
    out, _ = run_on_hw(x, Wq, Wk, Wv, Wo, bo)
    return out


# revision 11
# speedup vs baseline: 1.0001x; 1.0001x over previous
"""Graphformer layer (full multi-head attention) on 8 trn2 NeuronCores.

Sharding: one head per core (tensor parallel over the 8 heads).

v6 design (ACT-engine-bound, ~max overlap):
  - All matmul operands are bf16 (4x PE throughput vs fp32; tolerance is 2e-2).
  - Wo is folded into the V projection on the host: V' = x @ (Wo_h Wv_h)^T, so
    out_h = softmax(S) @ V' and no on-chip output projection is needed.
  - Scores are computed transposed (keys on partitions, queries on free dim).
    The K=64 contraction only fills half the PE array, so two key tiles are
    computed CONCURRENTLY via row-group packing: Q^T and K^T are materialized
    with duplicated partition halves (rows 0-63 == rows 64-127, free via
    host-duplicated projection weights); even/odd key-tile matmuls use
    partitions 0:64 / 64:128 -> PE row groups (0,0) / (64,0).
  - exp (scale=1/8 folded in) runs on the scalar engine in TRIPLES
    ([128, 3, 512] per instruction) from two alternating 3-bank PSUM score
    tensors.  Tile's PSUM dep tracking is per-tensor, so a single ring would
    serialize scores against exp; with A/B tensors exp(X) reads one tensor
    while triple X+1's scores write the other.  es (exp output) is split A/B
    for the same reason.  ACT is the bottleneck: ~131 us of back-to-back exp.
  - PV matmuls for exp-triple X are emitted AFTER the scores of triple X+1 so
    the strict-FIFO PE queue never blocks on the scalar engine.
  - Only K chunk 0 + Q chunk 0 are projected before the attention loop; the
    remaining K chunks, all V' batches and later Q chunks are interleaved
    into the attention stream through a single scratch PSUM bank ("pq"),
    paced to the prioritized input-DMA arrival order.
  - qT is split into even/odd-group tensors so the mid-group prefetch of the
    next Q chunk doesn't create a per-tensor false dependency on the current
    group's score matmuls.
  - PV accumulates [65, 512] per query group in PSUM; row 64 (ones column of
    V') is the softmax denominator.  Normalization, transpose, head-sum and
    bias happen on the host from the [65, 4096] per-core output.

PSUM budget (8 banks): scores A 3 + scores B 3 + PV accumulator 1 + scratch 1.
"""

from contextlib import ExitStack

import numpy as np
import ml_dtypes

import concourse.bass as bass
import concourse.bacc as bacc
import concourse.mybir as mybir
from concourse.tile import TileContext

N = 4096
C = 512  # input feature dim
D = 64  # head dim
Da = D + 1  # head dim + denominator column
HEADS = 8
P = 128
F = 512  # query group width (== one PSUM bank of fp32)
NT = N // P  # 32 key tiles
CT = C // P  # 4 contraction tiles
NG = N // F  # 8 query groups
F32 = mybir.dt.float32
BF16 = mybir.dt.bfloat16
BF16_NP = ml_dtypes.bfloat16


def build_nc():
    nc = bacc.Bacc()
    xT = nc.declare_dram_parameter("xT", [C, N], BF16, isOutput=False)
    # weights pre-arranged on host as [P, CT*w]: w2[p, c*w+m] = W^T[c*P+p, m]
    wqd = nc.declare_dram_parameter("wqd", [P, CT * P], BF16, isOutput=False)
    wkd = nc.declare_dram_parameter("wkd", [P, CT * P], BF16, isOutput=False)
    wv2 = nc.declare_dram_parameter("wv2", [P, CT * D], BF16, isOutput=False)
    ot = nc.declare_dram_parameter("ot", [Da, N], F32, isOutput=True)

    with TileContext(nc) as tc, ExitStack() as ctx:
        const = ctx.enter_context(tc.tile_pool(name="const", bufs=1))
        sb = ctx.enter_context(tc.tile_pool(name="sb", bufs=1))
        ot_pool = ctx.enter_context(tc.tile_pool(name="otp", bufs=2))
        ps_ring = ctx.enter_context(tc.tile_pool(name="psR", bufs=1, space="PSUM"))
        ps_a = ctx.enter_context(tc.tile_pool(name="psA", bufs=1, space="PSUM"))

        # ---- input DMAs, priority-ordered: weights + x cols 0:512 first
        # (those gate K0/Q0 and the start of the exp stream), then the rest of
        # x in arrival order of use.  Issue alternates sync/gpsimd sequencers.
        w_sb = {}
        for name, dram, w in (("k", wkd, P), ("q", wqd, P), ("v", wv2, D)):
            t = const.tile([P, CT, w], BF16, tag=f"w{name}")
            nc.gpsimd.dma_start(out=t, in_=dram[:, :])
            w_sb[name] = t
        xt = [
            sb.tile([P, N], BF16, tag=f"xt{c}", name=f"xt{c}") for c in range(CT)
        ]
        pieces = [(0, 512), (512, 1280), (1280, 2048), (2048, 4096)]
        n_issue = 0
        for lo, hi in pieces:
            for c in range(CT):
                eng = nc.sync if n_issue % 2 == 0 else nc.gpsimd
                n_issue += 1
                eng.dma_start(
                    out=xt[c][:, lo:hi], in_=xT[c * P : (c + 1) * P, lo:hi]
                )

        # persistent SBUF tensors
        qT = [
            sb.tile([P, N // 2], BF16, tag=f"qT{e}", name=f"qT{e}")
            for e in range(2)
        ]  # even/odd query groups, duplicated partition halves
        kT = sb.tile([P, N], BF16, tag="kT")  # duplicated halves
        v_sb = sb.tile([P, NT, Da], BF16, tag="v")
        es_a = sb.tile([P, 3, F], BF16, tag="esa")
        es_b = sb.tile([P, 3, F], BF16, tag="esb")
        es_ab = [es_a, es_b]
        ss_a = ps_ring.tile([P, 3, F], F32, tag="ssa")  # 3 banks
        ss_b = ps_ring.tile([P, 3, F], F32, tag="ssb")  # 3 banks
        ss_ab = [ss_a, ss_b]

        def s_slot(t):
            return ss_ab[(t // 3) % 2][:, t % 3, :]

        def es_slot(t):
            return es_ab[(t // 3) % 2][:, t % 3, :]

        def q_ap(g):
            return qT[g % 2][:, (g // 2) * F : (g // 2 + 1) * F]

        nc.vector.memset(v_sb[:, :, D:Da], 1.0)

        # warm the ACT exp table load while DMAs run
        dummy = const.tile([1, 1], F32, tag="dummy")
        nc.vector.memset(dummy, 0.0)
        nc.scalar.activation(
            out=dummy, in_=dummy, func=mybir.ActivationFunctionType.Exp
        )

        def pq_tile():
            return ps_a.tile([P, F], F32, tag="pq", name="pq")

        def k_chunk(ch):
            pp = pq_tile()
            cs = slice(ch * F, (ch + 1) * F)
            for c in range(CT):
                nc.tensor.matmul(
                    pp,
                    w_sb["k"][:, c, :],
                    xt[c][:, cs],
                    start=(c == 0),
                    stop=(c == CT - 1),
                )
            nc.vector.tensor_copy(out=kT[:, cs], in_=pp)

        def q_chunk(g):
            pp = pq_tile()
            cs = slice(g * F, (g + 1) * F)
            for c in range(CT):
                nc.tensor.matmul(
                    pp,
                    w_sb["q"][:, c, :],
                    xt[c][:, cs],
                    start=(c == 0),
                    stop=(c == CT - 1),
                )
            nc.vector.tensor_copy(out=q_ap(g), in_=pp)

        def v_batch(b):
            """Project 8 V' tiles (keys on partitions) via the scratch bank."""
            pv = pq_tile()
            for i in range(8):
                mt = b * 8 + i
                ms = slice(mt * P, (mt + 1) * P)
                for c in range(CT):
                    nc.tensor.matmul(
                        pv[:, i * D : (i + 1) * D],
                        xt[c][:, ms],
                        w_sb["v"][:, c, :],
                        start=(c == 0),
                        stop=(c == CT - 1),
                    )
            # free sizes match (512); element order (mt-major, then d) matches
            nc.vector.tensor_copy(out=v_sb[:, b * 8 : (b + 1) * 8, 0:D], in_=pv)

        # ---- prologue: only what gates the first exp
        k_chunk(0)
        q_chunk(0)

        # ---- attention over 256 score tiles, projections streamed in
        T = NG * NT  # 256
        po = [None] * NG
        pending_pv = []  # PV matmuls lagged one exp-triple behind scores

        def emit_pv(t):
            g, kt = divmod(t, NT)
            nc.tensor.matmul(
                po[g],
                v_sb[:, kt, :],
                es_slot(t),
                start=(kt == 0),
                stop=(kt == NT - 1),
            )
            if kt == NT - 1:
                ot_t = ot_pool.tile([Da, F], F32, tag="ot")
                for hh in range(2):
                    fs = slice(hh * (F // 2), (hh + 1) * (F // 2))
                    nc.vector.tensor_copy(out=ot_t[:, fs], in_=po[g][:, fs])
                    nc.sync.dma_start(
                        out=ot[
                            :, g * F + hh * (F // 2) : g * F + (hh + 1) * (F // 2)
                        ],
                        in_=ot_t[:, fs],
                    )

        def emit_exp(t0, cnt):
            ab = (t0 // 3) % 2
            nc.scalar.activation(
                out=es_ab[ab][:, 0:cnt, :],
                in_=ss_ab[ab][:, 0:cnt, :],
                func=mybir.ActivationFunctionType.Exp,
                scale=0.125,
            )
            pending_pv.extend(range(t0, t0 + cnt))

        def flush_pv():
            while pending_pv:
                emit_pv(pending_pv.pop(0))

        for t in range(T):
            g, kt = divmod(t, NT)
            if kt == 0:
                po[g] = ps_a.tile([Da, F], F32, tag="po", name=f"po{g}")
            # scores: even tiles use partitions 0:64 (PE rows 0-63), odd tiles
            # 64:128 (rows 64-127) -> consecutive matmuls run concurrently.
            h0 = (t % 2) * D
            hs = slice(h0, h0 + D)
            nc.tensor.matmul(
                s_slot(t),
                kT[hs, kt * P : (kt + 1) * P],
                q_ap(g)[hs, :],
                start=True,
                stop=True,
            )
            if t % 3 == 2:
                flush_pv()  # PVs of the previous triple, after this triple's S
                emit_exp(t - 2, 3)
            # streamed projections (each gates work >= 4 tiles ahead):
            if t % 4 == 0 and t // 4 + 1 < NG:
                k_chunk(t // 4 + 1)  # K c+1 emitted at t=4c: S(4c+4..) need it
            if t % 8 == 1 and t < 32:
                v_batch(t // 8)  # vb b at t=8b+1; first PV of tile 8b >= t=8b+4
            if kt == 16 and g + 1 < NG:
                q_chunk(g + 1)  # next group's Q, mid-group
        flush_pv()
        if T % 3:
            emit_exp(T - T % 3, T % 3)
            flush_pv()

    nc.compile()
    return nc


def make_in_maps(x, Wq, Wk, Wv, Wo):
    x = np.asarray(x, dtype=np.float32)
    Wq = np.asarray(Wq, dtype=np.float32)
    Wk = np.asarray(Wk, dtype=np.float32)
    Wv = np.asarray(Wv, dtype=np.float32)
    Wo = np.asarray(Wo, dtype=np.float32)
    xT = np.ascontiguousarray(x.T).astype(BF16_NP)

    def arrange(w):  # [C, width] -> [P, CT*width]: out[p, c*w+m] = w[c*P+p, m]
        width = w.shape[1]
        return np.ascontiguousarray(
            w.reshape(CT, P, width).transpose(1, 0, 2).reshape(P, CT * width)
        ).astype(BF16_NP)

    in_maps = []
    for h in range(HEADS):
        sl = slice(h * D, (h + 1) * D)
        wqT = Wq[sl].T
        wkT = Wk[sl].T
        wv2 = (Wo[:, sl] @ Wv[sl]).T  # [C, D], output projection folded in
        in_maps.append(
            {
                "xT": xT,
                "wqd": arrange(np.concatenate([wqT, wqT], axis=1)),
                "wkd": arrange(np.concatenate([wkT, wkT], axis=1)),
                "wv2": arrange(wv2),
            }
        )
    return in_maps


_CACHE = {}


def run_on_hw(x, Wq, Wk, Wv, Wo, bo, trace=False):
    from concourse.bass_utils import run_bass_kernel_spmd

    if "nc" not in _CACHE:
        _CACHE["nc"] = build_nc()
    nc = _CACHE["nc"]
    in_maps = make_in_maps(x, Wq, Wk, Wv, Wo)
    res = run_bass_kernel_spmd(nc, in_maps, list(range(HEADS)), trace=trace)
    out = np.zeros((N, D), np.float32)
    for r in res.results:
        o = r["ot"]
        out += (o[0:D, :] / o[D:Da, :]).T
    out += np.asarray(bo, dtype=np.float32)[None, :]
    return out, res


def kernel(x, Wq, Wk, Wv, Wo, bo):
    out, _ = run_on_hw(x, Wq, Wk, Wv, Wo, bo)
    return out
